# revision 19
# baseline (speedup 1.0000x reference)
"""Trainium2 Bass kernel for nn_MDR_local_global (hard-routed MoE + LN + dot logits).

Data-parallel over batch B=128 across 8 NeuronCores (16 batches/core).
The host pre-transposes log_feats to [ND, D, T] so the contraction dim loads
directly onto SBUF partitions (no on-chip transposes). Per core, per domain i
(4), tokens are processed in 25 tiles of 128:
  - 16 matmuls (4 experts x 4 k-subtiles) -> 4 PSUM banks (dtype selectable:
    fp32 / fp32r / bf16 via MDR_DT env, default fp32r)
  - disjoint-mask select sum_j proj_j*m_j fused into the PSUM drains
    (1 on ACT + 3 scalar_tensor_tensor on DVE, row-sum accumulated for free)
  - LayerNorm stats: E[x^2] via ACT Square+accum, normalize on ACT
  - embedding rows gathered by indirect DMA, logits via fused
    tensor_tensor_reduce on DVE
Outputs: mapped [4,3200,512] fp32, pos/neg logits [4,128(p),25(k)] fp32
(host de-interleaves the token tiling t = k*128 + p).
"""

import os

import numpy as np

ND, B, L, D, V = 4, 128, 200, 512, 100002
NCORES = 8
BSH = B // NCORES  # 16 batches per core
T = BSH * L        # 3200 tokens per domain per core
P = 128
KT = T // P        # 25 token tiles per domain
DK = D // P        # 4 contraction subtiles
LN_EPS = 1e-8

_BUILD_CACHE = {}


def _build_nc(dt_name, apply_gamma, apply_beta, apply_bias):
    import concourse.bass as bass
    import concourse.mybir as mybir
    import concourse.tile as tile
    from concourse import bacc
    from contextlib import ExitStack

    f32 = mybir.dt.float32
    i32 = mybir.dt.int32
    AL = mybir.AluOpType
    AF = mybir.ActivationFunctionType

    # dtype fed to the matmuls. float32r is bytewise fp32; declaring the DRAM
    # inputs as float32r keeps the BIR dtype chain consistent (the verifier
    # requires fp32r matmul operands to be produced as fp32r).
    if dt_name == "bf16":
        ST = mybir.dt.bfloat16
    elif dt_name == "fp32r":
        ST = mybir.dt.float32r
    else:
        ST = f32

    nc = bacc.Bacc()
    # lfT packed per tile: lfT[i, k, p, dk*P + t] = log_feats[i, k*P + t, dk*P + p]
    lfT = nc.dram_tensor("lfT", [ND, KT, P, DK * P], ST, kind="ExternalInput")
    # wt packed: wt[p, ((j*DK + dk)*D) + e] = W[j, e, dk*P + p]
    wt = nc.dram_tensor("wt", [P, ND * DK * D], ST, kind="ExternalInput")
    lg = nc.dram_tensor("lg", [P, ND, KT], i32, kind="ExternalInput")
    idxin = nc.dram_tensor("idxin", [P, 2, ND, KT], i32, kind="ExternalInput")
    emb = nc.dram_tensor("emb", [V, D], f32, kind="ExternalInput")
    gamma = nc.dram_tensor("gamma", [1, D], f32, kind="ExternalInput")
    beta = nc.dram_tensor("beta", [1, D], f32, kind="ExternalInput")
    bpad = nc.dram_tensor("bpad", [8, D], f32, kind="ExternalInput")  # row0=0, rows1..4=b_maps
    mapped = nc.dram_tensor("mapped", [ND, T, D], f32, kind="ExternalOutput")
    plog = nc.dram_tensor("plog", [ND, P, KT], f32, kind="ExternalOutput")
    nlog = nc.dram_tensor("nlog", [ND, P, KT], f32, kind="ExternalOutput")

    with tile.TileContext(nc) as tc, ExitStack() as ctx:
        consts = ctx.enter_context(tc.tile_pool(name="consts", bufs=1))
        xtp = ctx.enter_context(tc.tile_pool(name="xtp", bufs=3))
        drains = ctx.enter_context(tc.tile_pool(name="drains", bufs=3))
        emb_p = ctx.enter_context(tc.tile_pool(name="embp", bufs=3))
        outp = ctx.enter_context(tc.tile_pool(name="outp", bufs=3))
        stats = ctx.enter_context(tc.tile_pool(name="stats", bufs=4))
        logitp = ctx.enter_context(tc.tile_pool(name="logitp", bufs=2))
        trashp = ctx.enter_context(tc.tile_pool(name="trashp", bufs=2))
        psum_pr = ctx.enter_context(tc.tile_pool(name="pspr", bufs=1, space="PSUM"))

        wt_sb = consts.tile([P, ND, DK, D], ST)
        nc.sync.dma_start(
            out=wt_sb[:].rearrange("p j k e -> p (j k e)"), in_=wt[:]
        )

        eps_t = consts.tile([P, 1], f32)
        nc.vector.memset(eps_t[:], LN_EPS)

        if apply_gamma:
            gam_sb = consts.tile([P, D], f32)
            nc.gpsimd.dma_start(out=gam_sb[:], in_=gamma[:].to_broadcast((P, D)))
        if apply_beta:
            bet_sb = consts.tile([P, D], f32)
            nc.gpsimd.dma_start(out=bet_sb[:], in_=beta[:].to_broadcast((P, D)))

        # masks m_j(t) = (lg[t] == j+1), fp32, layout [P, i, k, j]
        masks = consts.tile([P, ND, KT, 4], f32)
        lg_sb = consts.tile([P, ND, KT], i32)
        nc.sync.dma_start(out=lg_sb[:], in_=lg[:])
        lgf = consts.tile([P, ND, KT], f32)
        nc.vector.tensor_copy(out=lgf[:], in_=lg_sb[:])
        for i in range(ND):
            for j in range(4):
                nc.vector.tensor_scalar(
                    out=masks[:, i, :, j],
                    in0=lgf[:, i, :],
                    scalar1=float(j + 1),
                    scalar2=None,
                    op0=AL.is_equal,
                )

        idxs = consts.tile([P, 2, ND, KT], i32)
        nc.sync.dma_start(out=idxs[:], in_=idxin[:])

        map_r = mapped[:].rearrange("i (k p) d -> i k p d", p=P)

        for i in range(ND):
            plog_sb = logitp.tile([P, KT], f32, tag="plog")
            nlog_sb = logitp.tile([P, KT], f32, tag="nlog")
            for k in range(KT):
                # x^T tile: [128(d), dk, 128(t)], host-packed contiguous
                xt_sb = xtp.tile([P, DK, P], ST, tag="xt")
                nc.scalar.dma_start(
                    out=xt_sb[:].rearrange("p a b -> p (a b)"), in_=lfT[i, k]
                )

                # 16 matmuls: psum_j += x^T[dk].T @ W^T[j,dk]
                prj = [
                    psum_pr.tile([P, D], f32, tag=f"prj{j}", name=f"prj{j}")
                    for j in range(4)
                ]
                for dk in range(DK):
                    lhsT = xt_sb[:, dk]
                    for j in range(4):
                        nc.tensor.matmul(
                            prj[j][:],
                            lhsT=lhsT,
                            rhs=wt_sb[:, j, dk],
                            start=(dk == 0),
                            stop=(dk == DK - 1),
                        )

                def m(j):
                    return masks[:, i, k, j:j + 1]

                # routed = sum_j proj_j * m_j  (disjoint masks)
                t0 = drains.tile([P, D], f32, tag="t0")
                nc.scalar.activation(out=t0[:], in_=prj[0][:], func=AF.Copy, scale=m(0))
                u = drains.tile([P, D], f32, tag="u")
                nc.vector.scalar_tensor_tensor(
                    out=u[:], in0=prj[1][:], scalar=m(1), in1=t0[:],
                    op0=AL.mult, op1=AL.add,
                )
                v = drains.tile([P, D], f32, tag="v")
                nc.vector.scalar_tensor_tensor(
                    out=v[:], in0=prj[2][:], scalar=m(2), in1=u[:],
                    op0=AL.mult, op1=AL.add,
                )
                r = drains.tile([P, D], f32, tag="r")
                sum_r = stats.tile([P, 1], f32, tag="sumr")
                nc.vector.scalar_tensor_tensor(
                    out=r[:], in0=prj[3][:], scalar=m(3), in1=v[:],
                    op0=AL.mult, op1=AL.add, accum_out=sum_r[:],
                )

                if apply_bias:
                    bias_sb = drains.tile([P, D], f32, tag="bias")
                    nc.gpsimd.indirect_dma_start(
                        out=bias_sb[:], out_offset=None, in_=bpad[:],
                        in_offset=bass.IndirectOffsetOnAxis(ap=lg_sb[:, i, k:k + 1], axis=0),
                    )
                    rb = drains.tile([P, D], f32, tag="rb")
                    sum_r = stats.tile([P, 1], f32, tag="sumrb")
                    nc.vector.scalar_tensor_tensor(
                        out=rb[:], in0=r[:], scalar=0.0, in1=bias_sb[:],
                        op0=AL.bypass, op1=AL.add, accum_out=sum_r[:],
                    )
                    r = rb

                # LN stats: sum(r^2) on ACT, tiny per-partition math on DVE
                sq = trashp.tile([P, D], f32, tag="sq")
                sum_r2 = stats.tile([P, 1], f32, tag="sumr2")
                nc.scalar.activation(out=sq[:], in_=r[:], func=AF.Square, accum_out=sum_r2[:])
                mu = stats.tile([P, 1], f32, tag="mu")
                nc.vector.tensor_scalar_mul(mu[:], sum_r[:], 1.0 / D)
                mu2 = stats.tile([P, 1], f32, tag="mu2")
                nc.vector.tensor_mul(mu2[:], mu[:], mu[:])
                var = stats.tile([P, 1], f32, tag="var")
                nc.vector.scalar_tensor_tensor(
                    out=var[:], in0=sum_r2[:], scalar=1.0 / D, in1=mu2[:],
                    op0=AL.mult, op1=AL.subtract,
                )
                sd = stats.tile([P, 1], f32, tag="sd")
                nc.scalar.activation(out=sd[:], in_=var[:], func=AF.Sqrt, bias=eps_t[:])
                rstd = stats.tile([P, 1], f32, tag="rstd")
                nc.vector.reciprocal(out=rstd[:], in_=sd[:])
                mneg = stats.tile([P, 1], f32, tag="mneg")
                nc.vector.scalar_tensor_tensor(
                    out=mneg[:], in0=mu[:], scalar=-1.0, in1=rstd[:],
                    op0=AL.mult, op1=AL.mult,
                )
                mp = outp.tile([P, D], f32, tag="mp")
                nc.scalar.activation(
                    out=mp[:], in_=r[:], func=AF.Identity, scale=rstd[:], bias=mneg[:]
                )
                if apply_gamma:
                    mpg = outp.tile([P, D], f32, tag="mpg")
                    nc.gpsimd.tensor_tensor(
                        out=mpg[:], in0=mp[:], in1=gam_sb[:], op=AL.mult,
                    )
                    mp = mpg
                if apply_beta:
                    mpb = outp.tile([P, D], f32, tag="mpb")
                    nc.gpsimd.tensor_tensor(
                        out=mpb[:], in0=mp[:], in1=bet_sb[:], op=AL.add,
                    )
                    mp = mpb

                # gather embedding rows; fused mult+rowsum for logits
                pe_sb = emb_p.tile([P, D], f32, tag="pe")
                nc.gpsimd.indirect_dma_start(
                    out=pe_sb[:], out_offset=None, in_=emb[:],
                    in_offset=bass.IndirectOffsetOnAxis(ap=idxs[:, 0, i, k:k + 1], axis=0),
                )
                ne_sb = emb_p.tile([P, D], f32, tag="ne")
                nc.gpsimd.indirect_dma_start(
                    out=ne_sb[:], out_offset=None, in_=emb[:],
                    in_offset=bass.IndirectOffsetOnAxis(ap=idxs[:, 1, i, k:k + 1], axis=0),
                )
                tr = trashp.tile([P, D], f32, tag="tr")
                sum_p = stats.tile([P, 1], f32, tag="sump")
                nc.vector.scalar_tensor_tensor(
                    out=tr[:], in0=mp[:], scalar=0.0, in1=pe_sb[:],
                    op0=AL.bypass, op1=AL.mult, accum_out=sum_p[:],
                )
                nc.vector.tensor_copy(out=plog_sb[:, k:k + 1], in_=sum_p[:])
                tr2 = trashp.tile([P, D], f32, tag="tr2")
                sum_n = stats.tile([P, 1], f32, tag="sumn")
                nc.vector.scalar_tensor_tensor(
                    out=tr2[:], in0=mp[:], scalar=0.0, in1=ne_sb[:],
                    op0=AL.bypass, op1=AL.mult, accum_out=sum_n[:],
                )
                nc.vector.tensor_copy(out=nlog_sb[:, k:k + 1], in_=sum_n[:])

                nc.scalar.dma_start(out=map_r[i, k], in_=mp[:])
            nc.scalar.dma_start(out=plog[i], in_=plog_sb[:])
            nc.scalar.dma_start(out=nlog[i], in_=nlog_sb[:])

    if not nc.is_finalized():
        nc.finalize()
    return nc


LAST_RESULT = None


def kernel(**inputs):
    global LAST_RESULT
    import ml_dtypes

    dt_name = os.environ.get("MDR_DT", "fp32r")

    lf = np.asarray(inputs["log_feats"], dtype=np.float32)
    W = np.asarray(inputs["W_maps"], dtype=np.float32)
    b = np.asarray(inputs["b_maps"], dtype=np.float32)
    emb = np.ascontiguousarray(np.asarray(inputs["emb_table"], dtype=np.float32))
    gam = np.asarray(inputs["ln_gamma"], dtype=np.float32)
    bet = np.asarray(inputs["ln_beta"], dtype=np.float32)
    lg = np.asarray(inputs["lg_dom"]).astype(np.int32)
    pos = np.asarray(inputs["pos_oth_dom"]).astype(np.int32)
    neg = np.asarray(inputs["neg_oth_dom"]).astype(np.int32)

    apply_gamma = not np.all(gam == 1.0)
    apply_beta = not np.all(bet == 0.0)
    apply_bias = not np.all(b == 0.0)

    key = (dt_name, apply_gamma, apply_beta, apply_bias)
    if key not in _BUILD_CACHE:
        _BUILD_CACHE[key] = _build_nc(*key)
    nc = _BUILD_CACHE[key]

    st_np = ml_dtypes.bfloat16 if dt_name == "bf16" else np.float32

    # wt[p, (j, dk, e)] = W[j, e, dk*128+p]
    wtv = W.transpose(0, 2, 1).reshape(ND, DK, P, D)  # [j, dk, p, e]
    wt = np.ascontiguousarray(wtv.transpose(2, 0, 1, 3).reshape(P, ND * DK * D)).astype(st_np)
    bpad = np.zeros((8, D), dtype=np.float32)
    bpad[1:5] = b
    gam2 = gam.reshape(1, D).copy()
    bet2 = bet.reshape(1, D).copy()

    def tokens_pk(a):
        # [BSH, ND, L] -> [P, ND, KT] with t = b*L + l = k*128 + p
        return a.transpose(1, 0, 2).reshape(ND, T).reshape(ND, KT, P).transpose(2, 0, 1)

    in_maps = []
    for c in range(NCORES):
        bs = slice(c * BSH, (c + 1) * BSH)
        # [ND, BSH, L, D] -> [ND, KT, P(d), DK, P(t)] packed tiles
        x = lf[:, bs].reshape(ND, KT, P, DK, P)  # [i, k, t, dk, p(d)]
        lfT_c = np.ascontiguousarray(
            x.transpose(0, 1, 4, 3, 2).reshape(ND, KT, P, DK * P)
        ).astype(st_np)
        idx2 = np.stack([tokens_pk(pos[bs]), tokens_pk(neg[bs])], axis=1)
        in_maps.append(
            dict(
                lfT=lfT_c,
                wt=wt,
                lg=np.ascontiguousarray(tokens_pk(lg[bs])),
                idxin=np.ascontiguousarray(idx2),
                emb=emb,
                gamma=gam2,
                beta=bet2,
                bpad=bpad,
            )
        )

    from concourse.bass_utils import run_bass_kernel_spmd

    trace = bool(int(os.environ.get("MDR_TRACE", "0")))
    try:
        LAST_RESULT = run_bass_kernel_spmd(
            nc, in_maps, core_ids=list(range(NCORES)), trace=trace
        )
    except ModuleNotFoundError:
        # axon client without the NTFF profile hook: rerun without tracing
        LAST_RESULT = run_bass_kernel_spmd(
            nc, in_maps, core_ids=list(range(NCORES)), trace=False
        )
    results = LAST_RESULT.results

    mapped = np.empty((ND, B, L, D), dtype=np.float32)
    plog_full = np.empty((ND, B, L), dtype=np.float32)
    nlog_full = np.empty((ND, B, L), dtype=np.float32)
    for c in range(NCORES):
        bs = slice(c * BSH, (c + 1) * BSH)
        mapped[:, bs] = results[c]["mapped"].reshape(ND, BSH, L, D)
        # [ND, P, KT] -> [ND, T] with t = k*128 + p
        for name, dst in (("plog", plog_full), ("nlog", nlog_full)):
            v = results[c][name].transpose(0, 2, 1).reshape(ND, T)
            dst[:, bs] = v.reshape(ND, BSH, L)
    return mapped, plog_full, nlog_full
